# revision 16
# baseline (speedup 1.0000x reference)
"""Multi-head self-attention Trainium2 kernel (8-core SPMD, head-parallel).

Problem: B=2, N=4096, D=768, H=12 heads, head_dim=64, fp32.

Sharding (Megatron-style tensor parallel over (batch, head) pairs):
  - 24 (b, h) pairs across 8 cores -> 3 heads per core, one batch per core
    (cores 0-3 -> batch 0 heads 0-11; cores 4-7 -> batch 1 heads 0-11).
  - Each core: QKV projection for its 3 heads, full attention for those
    heads, and a row-parallel slice of the output projection producing a
    *partial* [768, 4096] output (transposed layout).
  - Host sums the 4 partials per batch (the Megatron all-reduce), adds
    b_proj, transposes back.

On-core algorithm (per head):
  Q^T,K^T,V^T = W^T x^T (+bias)        [64, 4096] each, d on partitions
  V  = transpose(V^T) via PE           [4096, 64] in 32x[128,65] tiles,
                                       column 64 = 1.0 (row-sum trick)
  per 1024-query block, per 128-key chunk:
    S^T  = K^T_chunk.T @ Q^T_block     PSUM [128, 1024]  (fp32r matmul)
    P^T  = exp(SCALE * S^T)            ACT -> SBUF
    acc += V1_chunk.T @ P^T            PSUM [65, 1024], row 64 = softmax sums
  out^T = acc[0:64] * bcast(1/acc[64]) normalize via DVE + PE-ones broadcast
  partial^T += Wproj_h^T @ out^T       row-parallel output projection

Everything stays on-chip (SBUF/PSUM) between the x^T load and the final
partial^T store; no attention matrices ever touch HBM.  All matmuls run
in float32r (~12-bit mantissa, measured ~1.4e-4 relative per 128-dot);
the normalization broadcast uses exact fp32.
"""

import numpy as np

import concourse.bass as bass
import concourse.bacc as bacc
import concourse.mybir as mybir
import concourse.tile as tile
from concourse.bass_utils import run_bass_kernel_spmd

F32 = mybir.dt.float32
F32R = mybir.dt.float32r

B, N, D = 2, 4096, 768
H, HD = 12, 64
SCALE = HD ** -0.5
NCORES = 8
NH = 3            # heads per core
DC = D // 128     # 6 contraction chunks for the qkv projection
NB = N // 512     # 8 column blocks of 512
KC = N // 128     # 32 key chunks
QB = 1024         # query block (softmax/AV granularity)
NQB = N // QB     # 4 query blocks

# packed weight column layout (see _pack_core_inputs):
#   [qa(128) | ka(128) | qb(64) | kb(64) | va(128) | vb(64)]  -> 576 cols
_OFF_QA, _OFF_KA, _OFF_QB, _OFF_KB, _OFF_VA, _OFF_VB = 0, 128, 256, 320, 384, 512


def build_module() -> bass.Bass:
    nc = bacc.Bacc("TRN2", target_bir_lowering=False, debug=False)

    xT = nc.declare_dram_parameter("xT", [D, N], F32, isOutput=False)
    wp = nc.declare_dram_parameter("wp", [DC, 128, 576], F32, isOutput=False)
    bp = nc.declare_dram_parameter("bp", [128, 6], F32, isOutput=False)
    wproj = nc.declare_dram_parameter("wproj", [HD, NH, D], F32, isOutput=False)
    ident = nc.declare_dram_parameter("ident", [128, 128], F32, isOutput=False)
    outT = nc.declare_dram_parameter("outT", [D, N], F32, isOutput=True)

    with tile.TileContext(nc) as tc:
        with (
            tc.tile_pool(name="consts", bufs=1) as consts,
            tc.tile_pool(name="qkstore", bufs=1) as qkstore,
            tc.tile_pool(name="vstore", bufs=1) as vstore,
        ):
            # ---- persistent SBUF tensors -------------------------------
            w_sb = consts.tile([128, DC, 576], F32R)
            nc.sync.dma_start(w_sb[:], wp.rearrange("c p m -> p c m").bitcast(F32R))
            b_sb = consts.tile([128, 6], F32)
            nc.sync.dma_start(b_sb[:], bp[:])
            wproj_sb = consts.tile([HD, NH, D], F32R)
            nc.sync.dma_start(wproj_sb[:], wproj[:].bitcast(F32R))
            ident_sb = consts.tile([128, 128], F32)
            nc.sync.dma_start(ident_sb[:], ident[:])
            ones_sb = consts.tile([1, HD], F32)
            nc.vector.memset(ones_sb[:], 1.0)

            # Q^T/K^T: heads 0,1 packed on partitions [0:64]/[64:128] of the
            # "a" tiles; head 2 on partitions [0:64] of the "b" tiles.
            qTa = qkstore.tile([128, N], F32R)
            kTa = qkstore.tile([128, N], F32R)
            qTb = qkstore.tile([HD, N], F32R)
            kTb = qkstore.tile([HD, N], F32R)
            # V in [key, dim] layout, 65th column = 1.0 (row-sum trick)
            v_sb = [vstore.tile([128, KC, HD + 1], F32R, name=f"v_sb{h}") for h in range(NH)]
            for h in range(NH):
                nc.vector.memset(v_sb[h][:, :, HD:HD + 1].bitcast(F32), 1.0)

            # ---- phase 1: QKV projection + interleaved V transposes -----
            with (
                tc.tile_pool(name="xpool", bufs=3) as xpool,
                tc.tile_pool(name="vtpool", bufs=1) as vtpool,
                tc.tile_pool(name="prjpsum", bufs=4, space="PSUM") as prjpsum,
            ):
                vTa = vtpool.tile([128, N], F32)   # V^T heads 0,1
                vTb = vtpool.tile([HD, N], F32)    # V^T head 2
                groups = [
                    (qTa, _OFF_QA, 128, 0),
                    (kTa, _OFF_KA, 128, 1),
                    (qTb, _OFF_QB, HD, 2),
                    (kTb, _OFF_KB, HD, 3),
                    (vTa, _OFF_VA, 128, 4),
                    (vTb, _OFF_VB, HD, 5),
                ]
                with tc.tile_pool(name="tppsum", bufs=4, space="PSUM") as tppsum:
                    for nb in range(NB):
                        xt = xpool.tile([128, DC, 512], F32R)
                        nc.sync.dma_start(
                            xt[:],
                            xT.rearrange("(c p) n -> p c n", p=128)[
                                :, :, nb * 512:(nb + 1) * 512
                            ].bitcast(F32R),
                        )
                        for dest, off, m, bcol in groups:
                            pp = prjpsum.tile([128, 512], F32, tag="pp")
                            for c in range(DC):
                                nc.tensor.matmul(
                                    pp[0:m, :],
                                    w_sb[:, c, off:off + m],
                                    xt[:, c, :],
                                    start=(c == 0),
                                    stop=(c == DC - 1),
                                )
                            nc.vector.tensor_scalar_add(
                                dest[0:m, nb * 512:(nb + 1) * 512],
                                pp[0:m, :],
                                b_sb[0:m, bcol:bcol + 1],
                            )
                        # transpose this block's V^T columns into V tiles
                        # (interleaved to keep the PE stream dense)
                        for h in range(NH):
                            if h < 2:
                                src, base = vTa, 64 * h
                            else:
                                src, base = vTb, 0
                            for k in range(4 * nb, 4 * nb + 4):
                                tp = tppsum.tile([128, HD], F32, tag="tp")
                                nc.tensor.transpose(
                                    tp[:],
                                    src[base:base + HD, k * 128:(k + 1) * 128],
                                    ident_sb[base:base + HD, base:base + HD],
                                )
                                nc.vector.tensor_copy(v_sb[h][:, k, 0:HD], tp[:])

            # ---- phase 3: attention ------------------------------------
            with tc.tile_pool(name="attnstore", bufs=1) as attnstore:
                aT = [attnstore.tile([HD, N], F32R, name=f"aT{h}") for h in range(NH)]
                with (
                    tc.tile_pool(name="ppool", bufs=4) as ppool,
                    tc.tile_pool(name="upool", bufs=2) as upool,
                    tc.tile_pool(name="rpool", bufs=2) as rpool,
                    tc.tile_pool(name="spsum", bufs=3, space="PSUM") as spsum,
                    tc.tile_pool(name="avpsum", bufs=1, space="PSUM") as avpsum,
                ):
                    def qk_mm(kT, qT, base, q0):
                        # S^T chunk = K^T_chunk.T @ Q^T_block  -> PSUM
                        s = spsum.tile([128, QB], F32, tag="s", name="s")
                        k = qk_mm.k
                        qk_mm.k += 1
                        for x2 in range(QB // 512):
                            nc.tensor.matmul(
                                s[:, x2 * 512:(x2 + 1) * 512],
                                kT[base:base + HD, k * 128:(k + 1) * 128],
                                qT[base:base + HD, q0 + x2 * 512:q0 + (x2 + 1) * 512],
                                start=True,
                                stop=True,
                            )
                        return s

                    def emit_norm_tail(tail):
                        # deferred normalize finish: by now the reciprocal is
                        # long done, so the PE broadcast matmul doesn't stall
                        u65, r, dst = tail
                        bps = spsum.tile([HD, QB], F32, tag="s", name="bps")
                        for x2 in range(QB // 512):
                            nc.tensor.matmul(
                                bps[:, x2 * 512:(x2 + 1) * 512],
                                ones_sb[:],
                                r[:, x2 * 512:(x2 + 1) * 512],
                                start=True,
                                stop=True,
                            )
                        nc.vector.tensor_mul(dst, u65[0:HD, :], bps[:])

                    pending_tail = None
                    for h in range(NH):
                        if h < 2:
                            qT, kT, base = qTa, kTa, 64 * h
                        else:
                            qT, kT, base = qTb, kTb, 0
                        for qb in range(NQB):
                            q0 = qb * QB
                            av = avpsum.tile([HD + 1, QB], F32, tag="av", name="av")
                            # software pipeline: keep 3 S^T chunks in flight so
                            # the PE always has independent work during the exp
                            qk_mm.k = 0
                            s_tiles = [qk_mm(kT, qT, base, q0) for _ in range(3)]
                            for k in range(KC):
                                s = s_tiles.pop(0)
                                p = ppool.tile([128, QB], F32R, tag="p", name="p")
                                nc.scalar.activation(
                                    p[:], s[:], mybir.ActivationFunctionType.Exp,
                                    scale=SCALE,
                                )
                                # interleave the next chunk's QK halves with
                                # this chunk's AV halves: alternating stationary
                                # operands let each LDWEIGHTS prefetch into the
                                # PE background weight buffer during the other
                                # matmul's streaming
                                kn = qk_mm.k
                                do_prefetch = kn < KC
                                if do_prefetch:
                                    qk_mm.k += 1
                                    sn = spsum.tile([128, QB], F32, tag="s", name="s")
                                    s_tiles.append(sn)
                                for x2 in range(QB // 512):
                                    if do_prefetch:
                                        nc.tensor.matmul(
                                            sn[:, x2 * 512:(x2 + 1) * 512],
                                            kT[base:base + HD, kn * 128:(kn + 1) * 128],
                                            qT[base:base + HD, q0 + x2 * 512:q0 + (x2 + 1) * 512],
                                            start=True,
                                            stop=True,
                                        )
                                    nc.tensor.matmul(
                                        av[:, x2 * 512:(x2 + 1) * 512],
                                        v_sb[h][:, k, :],
                                        p[:, x2 * 512:(x2 + 1) * 512],
                                        start=(k == 0),
                                        stop=(k == KC - 1),
                                    )
                                if k == 8 and pending_tail is not None:
                                    emit_norm_tail(pending_tail)
                                    pending_tail = None
                            # normalize: out^T = av[0:64] / bcast(av[64]).
                            # One copy moves all 65 rows off PSUM (freeing the
                            # accumulator slot); the slow reciprocal then runs
                            # off the SBUF copy.
                            u65 = upool.tile([HD + 1, QB], F32, tag="u", name="u65")
                            nc.vector.tensor_copy(u65[:], av[:])
                            r = rpool.tile([1, QB], F32, tag="r", name="r")
                            nc.vector.reciprocal(r[:], u65[HD:HD + 1, :])
                            pending_tail = (u65, r, aT[h][:, q0:q0 + QB])
                    emit_norm_tail(pending_tail)

                # ---- phase 4: row-parallel output projection ------------
                with (
                    tc.tile_pool(name="opool", bufs=3) as opool,
                    tc.tile_pool(name="prpsum", bufs=4, space="PSUM") as prpsum,
                ):
                    for nb in range(NB):
                        for oc in range(DC):
                            pr = prpsum.tile([128, 512], F32, tag="pr")
                            for h in range(NH):
                                nc.tensor.matmul(
                                    pr[:],
                                    wproj_sb[:, h, oc * 128:(oc + 1) * 128],
                                    aT[h][:, nb * 512:(nb + 1) * 512],
                                    start=(h == 0),
                                    stop=(h == NH - 1),
                                )
                            ob = opool.tile([128, 512], F32, tag="ob")
                            nc.vector.tensor_copy(ob[:], pr[:])
                            nc.sync.dma_start(
                                outT[oc * 128:(oc + 1) * 128, nb * 512:(nb + 1) * 512],
                                ob[:],
                            )

    nc.compile()
    return nc


def _pack_core_inputs(core, x, W_qkv, b_qkv, W_proj):
    b = core // 4
    heads = [3 * (core % 4) + i for i in range(NH)]
    f32 = np.float32

    xT = np.ascontiguousarray(x[b].T, dtype=f32)

    def wcols(kind, h):  # kind 0=q 1=k 2=v
        return W_qkv[:, kind * D + h * HD: kind * D + (h + 1) * HD]

    wp_full = np.concatenate(
        [
            wcols(0, heads[0]), wcols(0, heads[1]),
            wcols(1, heads[0]), wcols(1, heads[1]),
            wcols(0, heads[2]), wcols(1, heads[2]),
            wcols(2, heads[0]), wcols(2, heads[1]),
            wcols(2, heads[2]),
        ],
        axis=1,
    )  # [768, 576]
    wp = np.ascontiguousarray(wp_full.reshape(DC, 128, 576), dtype=f32)

    def bcols(kind, h):
        return b_qkv[kind * D + h * HD: kind * D + (h + 1) * HD]

    z = np.zeros(HD, f32)
    bp_ = np.stack(
        [
            np.concatenate([bcols(0, heads[0]), bcols(0, heads[1])]),
            np.concatenate([bcols(1, heads[0]), bcols(1, heads[1])]),
            np.concatenate([bcols(0, heads[2]), z]),
            np.concatenate([bcols(1, heads[2]), z]),
            np.concatenate([bcols(2, heads[0]), bcols(2, heads[1])]),
            np.concatenate([bcols(2, heads[2]), z]),
        ],
        axis=1,
    ).astype(f32)  # [128, 6]

    wproj = np.stack(
        [W_proj[h * HD:(h + 1) * HD, :] for h in heads], axis=1
    ).astype(f32)  # [64, 3, 768]

    return {
        "xT": xT,
        "wp": wp,
        "bp": np.ascontiguousarray(bp_),
        "wproj": np.ascontiguousarray(wproj),
        "ident": np.eye(128, dtype=f32),
    }


_MODULE_CACHE = []


def _get_module() -> bass.Bass:
    if not _MODULE_CACHE:
        _MODULE_CACHE.append(build_module())
    return _MODULE_CACHE[0]


def kernel(x, W_qkv, b_qkv, W_proj, b_proj, _trace=False, _result_box=None):
    x = np.asarray(x, np.float32)
    W_qkv = np.asarray(W_qkv, np.float32)
    b_qkv = np.asarray(b_qkv, np.float32)
    W_proj = np.asarray(W_proj, np.float32)
    b_proj = np.asarray(b_proj, np.float32)

    nc = _get_module()
    in_maps = [
        _pack_core_inputs(c, x, W_qkv, b_qkv, W_proj) for c in range(NCORES)
    ]
    res = run_bass_kernel_spmd(nc, in_maps, list(range(NCORES)), trace=_trace)
    if _result_box is not None:
        _result_box.append(res)

    out = np.zeros((B, N, D), np.float32)
    for c in range(NCORES):
        out[c // 4] += res.results[c]["outT"].T
    out += b_proj
    return out
